# revision 4
# baseline (speedup 1.0000x reference)
"""Trainium2 Bass kernel for the DLI loss (ragged segment means -> pairwise NLL).

Math reduction: see _host_finish. Heavy work = ragged PREFIX sums of
encoder_output as a masked matmul P[T,D] = C[S,T]^T @ x[S,D] with
C[s,t] = (s <= end_t); the host takes adjacent differences to recover
segment sums. Data-parallel over 8 cores (4 batches each).

Design (v4, fp8 streaming at the HBM roofline):
- x quantized to fp8 e4m3 on the host (loss rel-err 5.9e-5, measured) and
  streamed at ~390-440 GB/s (measured) over the Sync-ring queue: 16-chunk
  1 MB tiles ([128, 16, 512] fp8) for batches 0-2 and a 16/8/4/4 taper for
  batch 3, so the PE (DoubleRow fp8, 259 ns/2-chunk steady) chases the
  stream and finishes each tile ~2 us after its last byte. Descriptor
  sizes 8/4/2 KB showed no rate loss vs 16 KB in the v3 trace.
- ends ride the Sync ring as the FIRST descriptor (the ACT/gpsimd queues
  have ~4-5 us first-transfer cold-start, measured), then a K=1 fp32
  ones-matmul broadcasts them to all 128 partitions; the is_le reads the
  broadcast straight out of PSUM.
- Dummy K=1 matmuls keep the PE busy between the broadcast and the first
  x tile: the PE p-state needs ~3 us of continuous work to reach 2.4 GHz
  (259 ns cadence), and at 1.2 GHz (454 ns) the PE cannot keep up with
  the stream.
- Prefix masks: one DVE is_le per batch (fp8 out, 2.75 us each), no
  subtract/copy. Position values come from tiny [P, 32, 1] iota columns
  read via stride-0 broadcast APs.
- ACT evacuates P per batch; output DMAs ride the warm Sync queue after
  all x triggers. Host: Q = P @ wlr in f64, A0/C0 = adjacent diffs.
"""

import sys
import os

sys.path.insert(0, "/opt/trn_rl_repo")

_jp = os.environ.get("JAX_PLATFORMS")
if _jp is not None and "axon" not in _jp and "jax" not in sys.modules:
    del os.environ["JAX_PLATFORMS"]

import numpy as np
import ml_dtypes

B, S, D, T = 32, 4096, 512, 64
N_CORES = 8
BPC = B // N_CORES          # batches per core
P = 128                     # SBUF partitions
NCH = S // P                # 32 chunks of [128, D] per batch

# (start_row, chunks, chunk_offset); s = row0 + p*ch + c_local.
STD_TILES = [(0, 16, 0), (2048, 16, 16)]
LAST_TILES = [(0, 16, 0), (2048, 8, 16), (3072, 4, 24), (3584, 4, 28)]

N_WARM_PRE = 5    # PE keep-busy matmuls before the ends broadcast
N_WARM_POST = 22  # ... and between the broadcast and the first x tile

_PROGRAM_CACHE = {}


def _build_program():
    from contextlib import ExitStack

    import concourse.bacc as bacc
    import concourse.mybir as mybir
    import concourse.tile as tile

    f32 = mybir.dt.float32
    fp8 = mybir.dt.float8e4

    nc = bacc.Bacc(
        "TRN2", target_bir_lowering=False, debug=False, enable_asserts=False
    )

    x_d = nc.dram_tensor("x", [BPC, S, D], fp8, kind="ExternalInput").ap()
    ends_d = nc.dram_tensor("endsb", [1, BPC * T], f32, kind="ExternalInput").ap()
    pfx_d = nc.dram_tensor("pfx", [BPC, T, D], f32, kind="ExternalOutput").ap()

    tilings = [STD_TILES] * (BPC - 1) + [LAST_TILES]

    with tile.TileContext(nc) as tc, ExitStack() as ctx:
        singles = ctx.enter_context(tc.tile_pool(name="singles", bufs=1))
        xpool = ctx.enter_context(tc.tile_pool(name="xp", bufs=1))
        mpool = ctx.enter_context(tc.tile_pool(name="mp", bufs=1))
        ppool = ctx.enter_context(tc.tile_pool(name="pp", bufs=1, space="PSUM"))

        # ends first on the Sync ring: 1 descriptor, lands with the queue's
        # first packet; every x descriptor queues behind it.
        ends_row = singles.tile([1, BPC * T], f32)
        nc.sync.dma_start(ends_row[:], ends_d)

        # x stream: dedicated slot per tile, all triggers queued upfront.
        xts = []
        for b in range(BPC):
            for t, (row0, ch, coff) in enumerate(tilings[b]):
                xt = xpool.tile([P, ch, D], fp8, tag=f"xt{b}_{t}", bufs=1)
                nc.sync.dma_start(
                    xt[:],
                    x_d[b][row0 : row0 + ch * P, :].rearrange(
                        "(p c) d -> p c d", c=ch
                    ),
                )
                xts.append(xt)

        ones_row = singles.tile([1, P], f32)
        nc.vector.memset(ones_row[:], 1.0)

        # PE p-state keep-busy + ends broadcast (K=1 fp32 matmuls).
        psum_w = ppool.tile([P, P], f32, tag="psw")
        for i in range(N_WARM_PRE):
            nc.tensor.matmul(
                psum_w[:], ones_row[:], ones_row[:], start=True, stop=True
            )
        psum_e = ppool.tile([P, BPC * T], f32, tag="pse")
        nc.tensor.matmul(psum_e[:], ones_row[:], ends_row[:], start=True, stop=True)
        for i in range(N_WARM_POST):
            nc.tensor.matmul(
                psum_w[:, : P // 2], ones_row[:], ones_row[:, : P // 2],
                start=True, stop=True,
            )
        ends_b = psum_e[:].rearrange("p (b t) -> p b t", b=BPC)

        # Position columns: value[p, c, 0] = row0 + p*ch + c_local per tile
        # layout; batches 0..2 share one column, batch 3 has its own.
        iota_t = singles.tile([P, NCH, 1], f32, tag="iota_t")
        for row0, ch, coff in STD_TILES:
            nc.gpsimd.iota(
                iota_t[:, coff : coff + ch, :],
                [[1, ch], [0, 1]],
                base=row0,
                channel_multiplier=ch,
                allow_small_or_imprecise_dtypes=True,
            )
        iota3 = singles.tile([P, NCH, 1], f32, tag="iota3")
        for row0, ch, coff in LAST_TILES:
            nc.gpsimd.iota(
                iota3[:, coff : coff + ch, :],
                [[1, ch], [0, 1]],
                base=row0,
                channel_multiplier=ch,
                allow_small_or_imprecise_dtypes=True,
            )

        # Prefix masks: cmpe[p,c,t] = (s <= end_t), fp8 {0,1}. One DVE op
        # per batch; ends read straight from PSUM.
        cmpes = []
        for b in range(BPC):
            col = iota3 if b == BPC - 1 else iota_t
            cmpe = mpool.tile([P, NCH, T], fp8, tag=f"cmpe{b}", bufs=1)
            nc.vector.tensor_tensor(
                cmpe[:],
                col[:].to_broadcast((P, NCH, T)),
                ends_b[:, b : b + 1, :].to_broadcast((P, NCH, T)),
                op=mybir.AluOpType.is_le,
            )
            cmpes.append(cmpe)

        # fp8 DoubleRow matmuls: 2 chunks per instruction. P evac on ACT,
        # output DMAs on the warm Sync queue (after all x triggers).
        for b in range(BPC):
            psum = ppool.tile([T, D], f32, tag=f"ps{b}")
            pair = 0
            xt_i = sum(len(tilings[bb]) for bb in range(b))
            for t, (row0, ch, coff) in enumerate(tilings[b]):
                xt = xts[xt_i + t]
                for j in range(ch // 2):
                    nc.tensor.matmul(
                        psum[:],
                        cmpes[b][:, coff + 2 * j : coff + 2 * j + 2, :],
                        xt[:, 2 * j : 2 * j + 2, :],
                        start=(pair == 0),
                        stop=(pair == NCH // 2 - 1),
                        perf_mode=mybir.MatmulPerfMode.DoubleRow,
                    )
                    pair += 1
            pfx_t = singles.tile([T, D], f32, tag=f"pfx{b}")
            nc.scalar.copy(pfx_t[:], psum[:])
            nc.sync.dma_start(pfx_d[b], pfx_t[:])

    nc.compile()
    return nc


def _host_prep(encoder_output, W, b, his_turn_end_ids):
    x = np.asarray(encoder_output, dtype=np.float32)
    xq = x.astype(ml_dtypes.float8_e4m3)
    W = np.asarray(W, dtype=np.float32)
    bias = np.asarray(b, dtype=np.float32)
    ends = np.asarray(his_turn_end_ids).astype(np.int64)

    ends_prev = np.concatenate(
        [np.full((B, 1), -1, np.int64), ends[:, :-1]], axis=1
    )
    endsb = ends.astype(np.float32)  # [B, T]

    wlr = np.stack([W[:D, 1] - W[:D, 0], W[D:, 1] - W[D:, 0]], axis=0)  # [2, D]
    bd = np.float64(np.float32(bias[1]) - np.float32(bias[0]))

    counts = (ends - ends_prev).astype(np.float64)  # [B, T]
    return xq, endsb, wlr, bd, counts


def _host_finish(A0, C0, counts, bd):
    A = A0 / counts
    C = C0 / counts
    u = A[:, :, None] + C[:, None, :] + bd  # [B, T, T]
    j = np.arange(T)[:, None]
    k = np.arange(T)[None, :]
    tri = k < j
    adj = k == (j - 1)
    nll = np.where(adj, np.logaddexp(0.0, -u), np.logaddexp(0.0, u))
    n_pairs = B * (T * (T - 1) // 2)
    loss = np.sum(np.where(tri, nll, 0.0)) / n_pairs
    return np.asarray(loss, dtype=np.float32)


def kernel(encoder_output, W, b, his_turn_end_ids):
    from concourse.bass_utils import run_bass_kernel_spmd

    xq, endsb, wlr, bd, counts = _host_prep(encoder_output, W, b, his_turn_end_ids)

    if "nc" not in _PROGRAM_CACHE:
        _PROGRAM_CACHE["nc"] = _build_program()
    nc = _PROGRAM_CACHE["nc"]

    in_maps = [
        {
            "x": xq[i * BPC : (i + 1) * BPC],
            "endsb": endsb[i * BPC : (i + 1) * BPC].reshape(1, BPC * T),
        }
        for i in range(N_CORES)
    ]
    trace = bool(int(os.environ.get("BASS_KERNEL_TRACE", "0")))
    kw = {}
    if os.environ.get("BASS_KERNEL_TMPDIR"):
        kw["tmpdir"] = os.environ["BASS_KERNEL_TMPDIR"]
    res = run_bass_kernel_spmd(nc, in_maps, list(range(N_CORES)), trace=trace, **kw)
    _PROGRAM_CACHE["last_results"] = res

    wlr64 = wlr.astype(np.float64)
    A0 = np.empty((B, T), np.float64)
    C0 = np.empty((B, T), np.float64)
    for i, r in enumerate(res.results):
        pfx = r["pfx"].astype(np.float64)  # [BPC, T, D]
        Q = pfx @ wlr64.T                  # [BPC, T, 2] prefix dots
        seg_dots = np.diff(Q, axis=1, prepend=0.0)  # [BPC, T, 2]
        A0[i * BPC : (i + 1) * BPC] = seg_dots[:, :, 0]
        C0[i * BPC : (i + 1) * BPC] = seg_dots[:, :, 1]
    return _host_finish(A0, C0, counts, bd)


# revision 6
# speedup vs baseline: 1.1621x; 1.1621x over previous
"""Trainium2 Bass kernel for the DLI loss (ragged segment means -> pairwise NLL).

Math reduction: see _host_finish. Heavy work = ragged PREFIX sums of
encoder_output as a masked matmul P[T,D] = C[S,T]^T @ x[S,D] with
C[s,t] = (s <= end_t); the host takes adjacent differences to recover
segment sums. Data-parallel over 8 cores (4 batches each).

Design (v4, fp8 streaming at the HBM roofline):
- x quantized to fp8 e4m3 on the host (loss rel-err 5.9e-5, measured) and
  streamed at ~390-440 GB/s (measured) over the Sync-ring queue: 16-chunk
  1 MB tiles ([128, 16, 512] fp8) for batches 0-2 and a 16/8/4/4 taper for
  batch 3, so the PE (DoubleRow fp8, 259 ns/2-chunk steady) chases the
  stream and finishes each tile ~2 us after its last byte. Descriptor
  sizes 8/4/2 KB showed no rate loss vs 16 KB in the v3 trace.
- ends ride the Sync ring as the FIRST descriptor (the ACT/gpsimd queues
  have ~4-5 us first-transfer cold-start, measured), then a K=1 fp32
  ones-matmul broadcasts them to all 128 partitions; the is_le reads the
  broadcast straight out of PSUM.
- Dummy K=1 matmuls keep the PE busy between the broadcast and the first
  x tile: the PE p-state needs ~3 us of continuous work to reach 2.4 GHz
  (259 ns cadence), and at 1.2 GHz (454 ns) the PE cannot keep up with
  the stream.
- Prefix masks: one DVE is_le per batch (fp8 out, 2.75 us each), no
  subtract/copy. Position values come from tiny [P, 32, 1] iota columns
  read via stride-0 broadcast APs.
- ACT evacuates P per batch; output DMAs ride the warm Sync queue after
  all x triggers. Host: Q = P @ wlr in f64, A0/C0 = adjacent diffs.
"""

import sys
import os

sys.path.insert(0, "/opt/trn_rl_repo")

_jp = os.environ.get("JAX_PLATFORMS")
if _jp is not None and "axon" not in _jp and "jax" not in sys.modules:
    del os.environ["JAX_PLATFORMS"]

import numpy as np
import ml_dtypes

B, S, D, T = 32, 4096, 512, 64
N_CORES = 8
BPC = B // N_CORES          # batches per core
P = 128                     # SBUF partitions
NCH = S // P                # 32 chunks of [128, D] per batch

# (start_row, chunks, chunk_offset); s = row0 + p*ch + c_local.
STD_TILES = [(0, 16, 0), (2048, 16, 16)]
LAST_TILES = [(0, 16, 0), (2048, 8, 16), (3072, 4, 24), (3584, 4, 28)]

N_WARM_POST = 10  # short bf16 PE keep-busy matmuls after the ends broadcast

_PROGRAM_CACHE = {}


def _build_program():
    from contextlib import ExitStack

    import concourse.bacc as bacc
    import concourse.mybir as mybir
    import concourse.tile as tile

    f32 = mybir.dt.float32
    fp8 = mybir.dt.float8e4

    nc = bacc.Bacc(
        "TRN2", target_bir_lowering=False, debug=False, enable_asserts=False
    )

    x_d = nc.dram_tensor("x", [BPC, S, D], fp8, kind="ExternalInput").ap()
    ends_d = nc.dram_tensor("endsb", [1, BPC * T], f32, kind="ExternalInput").ap()
    pfx_d = nc.dram_tensor("pfx", [BPC, T, D], f32, kind="ExternalOutput").ap()

    tilings = [STD_TILES] * (BPC - 1) + [LAST_TILES]

    with tile.TileContext(nc) as tc, ExitStack() as ctx:
        singles = ctx.enter_context(tc.tile_pool(name="singles", bufs=1))
        xpool = ctx.enter_context(tc.tile_pool(name="xp", bufs=1))
        mpool = ctx.enter_context(tc.tile_pool(name="mp", bufs=1))
        ppool = ctx.enter_context(tc.tile_pool(name="pp", bufs=1, space="PSUM"))

        # ends first on the Sync ring: 1 descriptor, lands with the queue's
        # first packet; every x descriptor queues behind it.
        ends_row = singles.tile([1, BPC * T], f32)
        nc.sync.dma_start(ends_row[:], ends_d)

        # x stream: dedicated slot per tile, all triggers queued upfront.
        xts = []
        for b in range(BPC):
            for t, (row0, ch, coff) in enumerate(tilings[b]):
                xt = xpool.tile([P, ch, D], fp8, tag=f"xt{b}_{t}", bufs=1)
                nc.sync.dma_start(
                    xt[:],
                    x_d[b][row0 : row0 + ch * P, :].rearrange(
                        "(p c) d -> p c d", c=ch
                    ),
                )
                xts.append(xt)

        ones_row = singles.tile([1, P], f32)
        nc.vector.memset(ones_row[:], 1.0)
        ones_bf = singles.tile([1, P], mybir.dt.bfloat16)
        nc.vector.memset(ones_bf[:], 1.0)

        # ends broadcast (K=1 fp32 matmul), then short bf16 dummies keep
        # the PE p-state ramping until the first x tile lands.
        psum_e = ppool.tile([P, BPC * T], f32, tag="pse")
        nc.tensor.matmul(psum_e[:], ones_row[:], ends_row[:], start=True, stop=True)
        psum_w = ppool.tile([P, P], f32, tag="psw")
        for i in range(N_WARM_POST):
            nc.tensor.matmul(
                psum_w[:], ones_bf[:], ones_bf[:], start=True, stop=True
            )
        ends_b = psum_e[:].rearrange("p (b t) -> p b t", b=BPC)

        # Position columns: value[p, c, 0] = row0 + p*ch + c_local per tile
        # layout; batches 0..2 share one column, batch 3 has its own.
        iota_t = singles.tile([P, NCH, 1], f32, tag="iota_t")
        for row0, ch, coff in STD_TILES:
            nc.gpsimd.iota(
                iota_t[:, coff : coff + ch, :],
                [[1, ch], [0, 1]],
                base=row0,
                channel_multiplier=ch,
                allow_small_or_imprecise_dtypes=True,
            )
        iota3 = singles.tile([P, NCH, 1], f32, tag="iota3")
        for row0, ch, coff in LAST_TILES:
            nc.gpsimd.iota(
                iota3[:, coff : coff + ch, :],
                [[1, ch], [0, 1]],
                base=row0,
                channel_multiplier=ch,
                allow_small_or_imprecise_dtypes=True,
            )

        # Prefix masks: cmpe[p,c,t] = (s <= end_t), fp8 {0,1}. One DVE op
        # per batch; ends read straight from PSUM.
        cmpes = []
        for b in range(BPC):
            col = iota3 if b == BPC - 1 else iota_t
            cmpe = mpool.tile([P, NCH, T], fp8, tag=f"cmpe{b}", bufs=1)
            nc.vector.tensor_tensor(
                cmpe[:],
                col[:].to_broadcast((P, NCH, T)),
                ends_b[:, b : b + 1, :].to_broadcast((P, NCH, T)),
                op=mybir.AluOpType.is_le,
            )
            cmpes.append(cmpe)

        # fp8 DoubleRow matmuls: 2 chunks per instruction. P evac on ACT,
        # output DMAs on the warm Sync queue (after all x triggers).
        for b in range(BPC):
            psum = ppool.tile([T, D], f32, tag=f"ps{b}")
            pair = 0
            xt_i = sum(len(tilings[bb]) for bb in range(b))
            for t, (row0, ch, coff) in enumerate(tilings[b]):
                xt = xts[xt_i + t]
                for j in range(ch // 2):
                    nc.tensor.matmul(
                        psum[:],
                        cmpes[b][:, coff + 2 * j : coff + 2 * j + 2, :],
                        xt[:, 2 * j : 2 * j + 2, :],
                        start=(pair == 0),
                        stop=(pair == NCH // 2 - 1),
                        perf_mode=mybir.MatmulPerfMode.DoubleRow,
                    )
                    pair += 1
            pfx_t = singles.tile([T, D], f32, tag=f"pfx{b}")
            nc.scalar.copy(pfx_t[:], psum[:])
            nc.sync.dma_start(pfx_d[b], pfx_t[:])

    nc.compile()
    return nc


def _host_prep(encoder_output, W, b, his_turn_end_ids):
    x = np.asarray(encoder_output, dtype=np.float32)
    xq = x.astype(ml_dtypes.float8_e4m3)
    W = np.asarray(W, dtype=np.float32)
    bias = np.asarray(b, dtype=np.float32)
    ends = np.asarray(his_turn_end_ids).astype(np.int64)

    ends_prev = np.concatenate(
        [np.full((B, 1), -1, np.int64), ends[:, :-1]], axis=1
    )
    endsb = ends.astype(np.float32)  # [B, T]

    wlr = np.stack([W[:D, 1] - W[:D, 0], W[D:, 1] - W[D:, 0]], axis=0)  # [2, D]
    bd = np.float64(np.float32(bias[1]) - np.float32(bias[0]))

    counts = (ends - ends_prev).astype(np.float64)  # [B, T]
    return xq, endsb, wlr, bd, counts


def _host_finish(A0, C0, counts, bd):
    A = A0 / counts
    C = C0 / counts
    u = A[:, :, None] + C[:, None, :] + bd  # [B, T, T]
    j = np.arange(T)[:, None]
    k = np.arange(T)[None, :]
    tri = k < j
    adj = k == (j - 1)
    nll = np.where(adj, np.logaddexp(0.0, -u), np.logaddexp(0.0, u))
    n_pairs = B * (T * (T - 1) // 2)
    loss = np.sum(np.where(tri, nll, 0.0)) / n_pairs
    return np.asarray(loss, dtype=np.float32)


def kernel(encoder_output, W, b, his_turn_end_ids):
    from concourse.bass_utils import run_bass_kernel_spmd

    xq, endsb, wlr, bd, counts = _host_prep(encoder_output, W, b, his_turn_end_ids)

    if "nc" not in _PROGRAM_CACHE:
        _PROGRAM_CACHE["nc"] = _build_program()
    nc = _PROGRAM_CACHE["nc"]

    in_maps = [
        {
            "x": xq[i * BPC : (i + 1) * BPC],
            "endsb": endsb[i * BPC : (i + 1) * BPC].reshape(1, BPC * T),
        }
        for i in range(N_CORES)
    ]
    trace = bool(int(os.environ.get("BASS_KERNEL_TRACE", "0")))
    kw = {}
    if os.environ.get("BASS_KERNEL_TMPDIR"):
        kw["tmpdir"] = os.environ["BASS_KERNEL_TMPDIR"]
    res = run_bass_kernel_spmd(nc, in_maps, list(range(N_CORES)), trace=trace, **kw)
    _PROGRAM_CACHE["last_results"] = res

    wlr64 = wlr.astype(np.float64)
    A0 = np.empty((B, T), np.float64)
    C0 = np.empty((B, T), np.float64)
    for i, r in enumerate(res.results):
        pfx = r["pfx"].astype(np.float64)  # [BPC, T, D]
        Q = pfx @ wlr64.T                  # [BPC, T, 2] prefix dots
        seg_dots = np.diff(Q, axis=1, prepend=0.0)  # [BPC, T, 2]
        A0[i * BPC : (i + 1) * BPC] = seg_dots[:, :, 0]
        C0[i * BPC : (i + 1) * BPC] = seg_dots[:, :, 1]
    return _host_finish(A0, C0, counts, bd)


# revision 8
# speedup vs baseline: 1.1653x; 1.0027x over previous
"""Trainium2 Bass kernel for the DLI loss (ragged segment means -> pairwise NLL).

Math reduction: see _host_finish. Heavy work = ragged PREFIX sums of
encoder_output as a masked matmul P[T,D] = C[S,T]^T @ x[S,D] with
C[s,t] = (s <= end_t); the host takes adjacent differences to recover
segment sums. Data-parallel over 8 cores (4 batches each).

Design (v4, fp8 streaming at the HBM roofline):
- x quantized to fp8 e4m3 on the host (loss rel-err 5.9e-5, measured) and
  streamed at ~390-440 GB/s (measured) over the Sync-ring queue: 16-chunk
  1 MB tiles ([128, 16, 512] fp8) for batches 0-2 and a 16/8/4/4 taper for
  batch 3, so the PE (DoubleRow fp8, 259 ns/2-chunk steady) chases the
  stream and finishes each tile ~2 us after its last byte. Descriptor
  sizes 8/4/2 KB showed no rate loss vs 16 KB in the v3 trace.
- ends ride the Sync ring as the FIRST descriptor (the ACT/gpsimd queues
  have ~4-5 us first-transfer cold-start, measured), then a K=1 fp32
  ones-matmul broadcasts them to all 128 partitions; the is_le reads the
  broadcast straight out of PSUM.
- Dummy K=1 matmuls keep the PE busy between the broadcast and the first
  x tile: the PE p-state needs ~3 us of continuous work to reach 2.4 GHz
  (259 ns cadence), and at 1.2 GHz (454 ns) the PE cannot keep up with
  the stream.
- Prefix masks: one DVE is_le per batch (fp8 out, 2.75 us each), no
  subtract/copy. Position values come from tiny [P, 32, 1] iota columns
  read via stride-0 broadcast APs.
- ACT evacuates P per batch; output DMAs ride the warm Sync queue after
  all x triggers. Host: Q = P @ wlr in f64, A0/C0 = adjacent diffs.
"""

import sys
import os

sys.path.insert(0, "/opt/trn_rl_repo")

_jp = os.environ.get("JAX_PLATFORMS")
if _jp is not None and "axon" not in _jp and "jax" not in sys.modules:
    del os.environ["JAX_PLATFORMS"]

import numpy as np
import ml_dtypes

B, S, D, T = 32, 4096, 512, 64
N_CORES = 8
BPC = B // N_CORES          # batches per core
P = 128                     # SBUF partitions
NCH = S // P                # 32 chunks of [128, D] per batch

# (start_row, chunks, chunk_offset); s = row0 + p*ch + c_local.
STD_TILES = [(0, 16, 0), (2048, 16, 16)]
LAST_TILES = [(0, 16, 0), (2048, 8, 16), (3072, 4, 24), (3584, 2, 28), (3840, 2, 30)]

N_WARM_POST = 14  # short bf16 PE keep-busy matmuls after the ends broadcast

_PROGRAM_CACHE = {}


def _build_program():
    from contextlib import ExitStack

    import concourse.bacc as bacc
    import concourse.mybir as mybir
    import concourse.tile as tile

    f32 = mybir.dt.float32
    fp8 = mybir.dt.float8e4

    nc = bacc.Bacc(
        "TRN2", target_bir_lowering=False, debug=False, enable_asserts=False
    )

    x_d = nc.dram_tensor("x", [BPC, S, D], fp8, kind="ExternalInput").ap()
    ends_d = nc.dram_tensor("endsb", [1, BPC * T], f32, kind="ExternalInput").ap()
    pfx_d = nc.dram_tensor("pfx", [BPC, T, D], f32, kind="ExternalOutput").ap()

    tilings = [STD_TILES] * (BPC - 1) + [LAST_TILES]

    with tile.TileContext(nc) as tc, ExitStack() as ctx:
        singles = ctx.enter_context(tc.tile_pool(name="singles", bufs=1))
        xpool = ctx.enter_context(tc.tile_pool(name="xp", bufs=1))
        mpool = ctx.enter_context(tc.tile_pool(name="mp", bufs=1))
        ppool = ctx.enter_context(tc.tile_pool(name="pp", bufs=1, space="PSUM"))

        # ends first on the Sync ring: 1 descriptor, lands with the queue's
        # first packet; every x descriptor queues behind it.
        ends_row = singles.tile([1, BPC * T], f32)
        nc.sync.dma_start(ends_row[:], ends_d)

        # x stream: dedicated slot per tile, all triggers queued upfront.
        xts = []
        for b in range(BPC):
            for t, (row0, ch, coff) in enumerate(tilings[b]):
                xt = xpool.tile([P, ch, D], fp8, tag=f"xt{b}_{t}", bufs=1)
                nc.sync.dma_start(
                    xt[:],
                    x_d[b][row0 : row0 + ch * P, :].rearrange(
                        "(p c) d -> p c d", c=ch
                    ),
                )
                xts.append(xt)

        ones_row = singles.tile([1, P], f32)
        nc.vector.memset(ones_row[:], 1.0)
        ones_bf = singles.tile([1, P], mybir.dt.bfloat16)
        nc.vector.memset(ones_bf[:], 1.0)

        # ends broadcast (K=1 fp32 matmul), then short bf16 dummies keep
        # the PE p-state ramping until the first x tile lands.
        psum_e = ppool.tile([P, BPC * T], f32, tag="pse")
        nc.tensor.matmul(psum_e[:], ones_row[:], ends_row[:], start=True, stop=True)
        psum_w = ppool.tile([P, P], f32, tag="psw")
        for i in range(N_WARM_POST):
            nc.tensor.matmul(
                psum_w[:], ones_bf[:], ones_bf[:], start=True, stop=True
            )
        ends_b = psum_e[:].rearrange("p (b t) -> p b t", b=BPC)

        # Position columns: value[p, c, 0] = row0 + p*ch + c_local per tile
        # layout; batches 0..2 share one column, batch 3 has its own.
        iota_t = singles.tile([P, NCH, 1], f32, tag="iota_t")
        for row0, ch, coff in STD_TILES:
            nc.gpsimd.iota(
                iota_t[:, coff : coff + ch, :],
                [[1, ch], [0, 1]],
                base=row0,
                channel_multiplier=ch,
                allow_small_or_imprecise_dtypes=True,
            )
        iota3 = singles.tile([P, NCH, 1], f32, tag="iota3")
        for row0, ch, coff in LAST_TILES:
            nc.gpsimd.iota(
                iota3[:, coff : coff + ch, :],
                [[1, ch], [0, 1]],
                base=row0,
                channel_multiplier=ch,
                allow_small_or_imprecise_dtypes=True,
            )

        # Prefix masks: cmpe[p,c,t] = (s <= end_t), fp8 {0,1}. One DVE op
        # per batch; ends read straight from PSUM.
        cmpes = []
        for b in range(BPC):
            col = iota3 if b == BPC - 1 else iota_t
            cmpe = mpool.tile([P, NCH, T], fp8, tag=f"cmpe{b}", bufs=1)
            nc.vector.tensor_tensor(
                cmpe[:],
                col[:].to_broadcast((P, NCH, T)),
                ends_b[:, b : b + 1, :].to_broadcast((P, NCH, T)),
                op=mybir.AluOpType.is_le,
            )
            cmpes.append(cmpe)

        # fp8 DoubleRow matmuls: 2 chunks per instruction. P evac on ACT,
        # output DMAs on the warm Sync queue (after all x triggers).
        for b in range(BPC):
            psum = ppool.tile([T, D], f32, tag=f"ps{b}")
            pair = 0
            xt_i = sum(len(tilings[bb]) for bb in range(b))
            for t, (row0, ch, coff) in enumerate(tilings[b]):
                xt = xts[xt_i + t]
                for j in range(ch // 2):
                    nc.tensor.matmul(
                        psum[:],
                        cmpes[b][:, coff + 2 * j : coff + 2 * j + 2, :],
                        xt[:, 2 * j : 2 * j + 2, :],
                        start=(pair == 0),
                        stop=(pair == NCH // 2 - 1),
                        perf_mode=mybir.MatmulPerfMode.DoubleRow,
                    )
                    pair += 1
            pfx_t = singles.tile([T, D], f32, tag=f"pfx{b}")
            nc.scalar.copy(pfx_t[:], psum[:])
            # ACT-ring trigger: its queue is otherwise empty, so these
            # descriptors race ahead of the q=1 end-of-stream stragglers.
            nc.scalar.dma_start(pfx_d[b], pfx_t[:])

    nc.compile()
    return nc


def _host_prep(encoder_output, W, b, his_turn_end_ids):
    x = np.asarray(encoder_output, dtype=np.float32)
    xq = x.astype(ml_dtypes.float8_e4m3)
    W = np.asarray(W, dtype=np.float32)
    bias = np.asarray(b, dtype=np.float32)
    ends = np.asarray(his_turn_end_ids).astype(np.int64)

    ends_prev = np.concatenate(
        [np.full((B, 1), -1, np.int64), ends[:, :-1]], axis=1
    )
    endsb = ends.astype(np.float32)  # [B, T]

    wlr = np.stack([W[:D, 1] - W[:D, 0], W[D:, 1] - W[D:, 0]], axis=0)  # [2, D]
    bd = np.float64(np.float32(bias[1]) - np.float32(bias[0]))

    counts = (ends - ends_prev).astype(np.float64)  # [B, T]
    return xq, endsb, wlr, bd, counts


def _host_finish(A0, C0, counts, bd):
    A = A0 / counts
    C = C0 / counts
    u = A[:, :, None] + C[:, None, :] + bd  # [B, T, T]
    j = np.arange(T)[:, None]
    k = np.arange(T)[None, :]
    tri = k < j
    adj = k == (j - 1)
    nll = np.where(adj, np.logaddexp(0.0, -u), np.logaddexp(0.0, u))
    n_pairs = B * (T * (T - 1) // 2)
    loss = np.sum(np.where(tri, nll, 0.0)) / n_pairs
    return np.asarray(loss, dtype=np.float32)


def kernel(encoder_output, W, b, his_turn_end_ids):
    from concourse.bass_utils import run_bass_kernel_spmd

    xq, endsb, wlr, bd, counts = _host_prep(encoder_output, W, b, his_turn_end_ids)

    if "nc" not in _PROGRAM_CACHE:
        _PROGRAM_CACHE["nc"] = _build_program()
    nc = _PROGRAM_CACHE["nc"]

    in_maps = [
        {
            "x": xq[i * BPC : (i + 1) * BPC],
            "endsb": endsb[i * BPC : (i + 1) * BPC].reshape(1, BPC * T),
        }
        for i in range(N_CORES)
    ]
    trace = bool(int(os.environ.get("BASS_KERNEL_TRACE", "0")))
    kw = {}
    if os.environ.get("BASS_KERNEL_TMPDIR"):
        kw["tmpdir"] = os.environ["BASS_KERNEL_TMPDIR"]
    res = run_bass_kernel_spmd(nc, in_maps, list(range(N_CORES)), trace=trace, **kw)
    _PROGRAM_CACHE["last_results"] = res

    wlr64 = wlr.astype(np.float64)
    A0 = np.empty((B, T), np.float64)
    C0 = np.empty((B, T), np.float64)
    for i, r in enumerate(res.results):
        pfx = r["pfx"].astype(np.float64)  # [BPC, T, D]
        Q = pfx @ wlr64.T                  # [BPC, T, 2] prefix dots
        seg_dots = np.diff(Q, axis=1, prepend=0.0)  # [BPC, T, 2]
        A0[i * BPC : (i + 1) * BPC] = seg_dots[:, :, 0]
        C0[i * BPC : (i + 1) * BPC] = seg_dots[:, :, 1]
    return _host_finish(A0, C0, counts, bd)
